# revision 28
# baseline (speedup 1.0000x reference)
"""Trainium2 Bass kernel for BEiT attention block (nn_Beit_9560597201107).

Data-parallel over batch: 64 batches -> 8 NeuronCores x 8 batches each.
Fully transposed dataflow (channels on partitions) so the softmax'd
attention matrix is never transposed on-chip:

  xT = x.T (host)                                  [768, 197]
  qkT[c, n] = sum_k WT[k, c] xT[k, n] + bias       [1536, 197]  (q pre-scaled)
  v[m, d]   = sum_k xT[k, m] WT_v[k, d] + bias     [197, 768]   (natural)
  scT[m, n] = sum_d kT[d, m] qT[d, n]              per head
  eT = exp(scT) * exp_rel_T                        (rel bias via exp-mult)
  po[d, n], sums[n] = sum_m [v|1][m, d] eT[m, n]   (ones col -> row 64 = sums)
  cT = po * broadcast(1/sums)   (PE ones-outer-product broadcast)
  y[n, o] = sum_c cT[c, n] projWT[c, o] + bias

Matmuls run in bfloat16 (fp32 PSUM accumulation, 1 cycle/row at any free
size), except the big qk projection which uses fp8e4 with DoubleRow perf
mode (0.5 cycles/row, half the PE energy -> less power throttling).  fp8
q/k weights are pre-scaled x64 on the host to stay out of the subnormal
range and rescaled by 1/64 in the PSUM->SBUF copy.  The attention
normalization is deferred two head-pair steps and the scores of the next
pair are prefetched so the PE never waits on the exp/reciprocal chains.
"""

import os
import numpy as np
import ml_dtypes

import concourse.bass as bass
import concourse.bacc as bacc
import concourse.mybir as mybir
import concourse.tile as tile
from concourse.bass_utils import run_bass_kernel_spmd
from concourse.bass_interp import get_hw_module

B, N, DIM, HEADS, NBS = 64, 197, 768, 12, 10
HEAD_DIM = DIM // HEADS
SCALE = HEAD_DIM ** -0.5
NCORES = 8
BPC = B // NCORES          # batches per core
KT = DIM // 128            # 6 contraction tiles
NF = N                     # token free-dim, exact (bf16 full rate at any size)
NF2 = 2 * NF
TOK_TILES = [(0, 128), (128, 69)]  # (offset, size) over the 197 tokens
# Heads grouped in same-parity pairs: both heads of a pair live at the same
# 64-partition half of qkT, so their back-to-back matmuls into one PSUM bank
# use the same PE row group.
PAIRS = [(0, 2), (4, 6), (8, 10), (1, 3), (5, 7), (9, 11)]
PAIR_PERM = [h for p in PAIRS for h in p]

F32 = mybir.dt.float32
FP8 = mybir.dt.float8e4
FP8NP = ml_dtypes.float8_e4m3
QS = 64.0  # fp8 weight pre-scale (keeps q/k weights out of subnormal range)
BF16 = mybir.dt.bfloat16
BFNP = ml_dtypes.bfloat16

_CACHE = {}


def _build_module():
    nc = bacc.Bacc("TRN2", target_bir_lowering=False, debug=False)

    # host-transposed x: xt8[b, k, p, n] = x[b, n, 128k+p]
    xt8_d = nc.dram_tensor("xt8", [BPC, 128, KT, NF], BF16, kind="ExternalInput")
    wt_d = nc.dram_tensor("wt", [KT, 128, DIM], BF16, kind="ExternalInput")
    wt8_d = nc.dram_tensor("wt8", [3, 128, 2, 2 * DIM], FP8, kind="ExternalInput")
    xt8f8_d = nc.dram_tensor("xt8f8", [BPC // 2, 128, 3, 2, 2, NF], FP8,
                             kind="ExternalInput")
    pwt_d = nc.dram_tensor("pwt", [128, KT, DIM], BF16, kind="ExternalInput")
    qbc_d = nc.dram_tensor("qbc", [128, BPC, KT], F32, kind="ExternalInput")
    vpb_d = nc.dram_tensor("vpb8", [BPC, 128, DIM], BF16, kind="ExternalInput")
    relt_d = nc.dram_tensor("relt", [128, 6, 2, NF2], BF16, kind="ExternalInput")
    twohot_d = nc.dram_tensor("twohot", [128, 128], BF16, kind="ExternalInput")
    y8_d = nc.dram_tensor("y8", [BPC, N, DIM], F32, kind="ExternalOutput")

    with tile.TileContext(nc) as tc:
        with (
            tc.tile_pool(name="const", bufs=1) as constp,
            tc.tile_pool(name="sb_xT", bufs=4) as sb_xT,
            tc.tile_pool(name="sb_qkT", bufs=2) as sb_qkT,
            tc.tile_pool(name="sb_v", bufs=2) as sb_v,
            tc.tile_pool(name="sb_exp", bufs=2) as sb_exp,
            tc.tile_pool(name="sb_po", bufs=3) as sb_po,
            tc.tile_pool(name="sb_pohi", bufs=3) as sb_pohi,
            tc.tile_pool(name="sb_rec", bufs=6) as sb_rec,
            tc.tile_pool(name="sb_cT", bufs=12) as sb_cT,
            tc.tile_pool(name="sb_out", bufs=2) as sb_out,
            tc.tile_pool(name="sb_vpb", bufs=2) as sb_vpb,
            tc.tile_pool(name="ps", bufs=5, space="PSUM") as ps,
            tc.tile_pool(name="ps_pj", bufs=2, space="PSUM") as ps_pj,
            tc.tile_pool(name="ps_pb", bufs=1, space="PSUM") as ps_pb,
        ):
            # ---- persistent data; wt chunked per k-tile so the first qkT
            # ---- matmuls can start as soon as chunk 0 lands
            qbc_sb = constp.tile([128, BPC, KT], F32)
            nc.sync.dma_start(out=qbc_sb[:], in_=qbc_d.ap())
            wt8_sb = constp.tile([128, 3, 2, 2 * DIM], FP8)
            nc.gpsimd.dma_start(out=wt8_sb[:, 0, :, 0:512],
                                in_=wt8_d.ap()[0][:, :, 0:512])
            nc.gpsimd.dma_start(out=wt8_sb[:, 0, :, 512:2 * DIM],
                                in_=wt8_d.ap()[0][:, :, 512:2 * DIM])
            for k in range(1, 3):
                nc.gpsimd.dma_start(out=wt8_sb[:, k, :, :], in_=wt8_d.ap()[k])
            wt_sb = constp.tile([128, KT, DIM], BF16)
            for k in range(KT):
                nc.gpsimd.dma_start(out=wt_sb[:, k, :], in_=wt_d.ap()[k])
            relt_sb = constp.tile([128, 6, 2, NF2], BF16)
            nc.gpsimd.dma_start(out=relt_sb[:], in_=relt_d.ap())
            pwt_sb = constp.tile([128, KT, DIM], BF16)
            nc.gpsimd.dma_start(out=pwt_sb[:], in_=pwt_d.ap())
            twohot_sb = constp.tile([128, 128], BF16)
            nc.gpsimd.dma_start(out=twohot_sb[:], in_=twohot_d.ap())

            def kT(qkT_sb, h, hb, off, mt):
                base = (h % 2) * 64
                return qkT_sb[base:base + 64, 6 + h // 2,
                              hb * NF + off:hb * NF + off + mt]

            def qT(qkT_sb, h, hb):
                base = (h % 2) * 64
                return qkT_sb[base:base + 64, h // 2, hb * NF:(hb + 1) * NF]

            # ---- projection machinery (generic over prev/self batch) ----
            def make_pstate(b_, cT_, vpb_):
                return {"b": b_, "cT": cT_, "vpb": vpb_, "ps": {}}

            def emit_pj(st, t, js):
                off, mt = TOK_TILES[t]
                if t not in st["ps"]:
                    st["ps"][t] = (
                        ps_pj.tile([128, 512], F32, tag="pj",
                                   name=f"pr_{st['b']}_{t}"),
                        ps_pj.tile([128, 256], F32, tag="pj",
                                   name=f"pr2_{st['b']}_{t}"),
                    )
                pr, pr2 = st["ps"][t]
                for j in js:
                    nc.tensor.matmul(
                        pr[0:mt, :], st["cT"][j][:, off:off + mt],
                        pwt_sb[:, j, 0:512], start=(j == 0), stop=(j == 5),
                    )
                    nc.tensor.matmul(
                        pr2[0:mt, :], st["cT"][j][:, off:off + mt],
                        pwt_sb[:, j, 512:768], start=(j == 0), stop=(j == 5),
                    )

            def emit_pj_fin(st, t):
                off, mt = TOK_TILES[t]
                pr, pr2 = st["ps"][t]
                out_sb = sb_out.tile([128, DIM], F32, tag="out",
                                     name=f"out_{st['b']}_{t}")
                nc.vector.tensor_add(out_sb[0:mt, 0:512], pr[0:mt, :],
                                     st["vpb"][0:mt, 0:512])
                nc.vector.tensor_add(out_sb[0:mt, 512:768], pr2[0:mt, :],
                                     st["vpb"][0:mt, 512:768])
                nc.sync.dma_start(out=y8_d.ap()[st["b"], off:off + mt, :],
                                  in_=out_sb[0:mt, :])

            prev_proj = [None]
            pend_norm = [None]

            for g in range(BPC // 2):
                # ---- load host-transposed x for the batch pair (sync queue
                # ---- so it runs in parallel with the const loads) ----
                xT8_sb = sb_xT.tile([128, 3, 2, 2, NF], FP8, tag="xT8",
                                    name=f"xT8_{g}")
                for kk in range(3):
                    nc.gpsimd.dma_start(out=xT8_sb[:, kk, :, :, :],
                                        in_=xt8f8_d.ap()[g][:, kk])
                xT_sb = sb_xT.tile([128, 2, KT, NF], BF16, tag="xT", name=f"xT_{g}")
                for hb in range(2):
                    nc.gpsimd.dma_start(
                        out=xT_sb[:, hb, :, :],
                        in_=xt8_d.ap()[2 * g + hb],
                    )

                # ---- qkT for both batches; k-outer in two ct-halves so the
                # ---- first matmuls only need wt chunk 0 ----
                qkT_sb = sb_qkT.tile([128, 12, NF2], BF16, tag="qkT", name=f"qkT_{g}")
                for third in range(3):
                    cts = list(range(4 * third, 4 * third + 4))
                    qps = {ct: ps.tile([128, NF2], F32, tag="ps",
                                       name=f"qp_{g}_{ct}") for ct in cts}
                    for k in range(3):
                        for ct in cts:
                            nc.tensor.matmul(
                                qps[ct][:],
                                wt8_sb[:, k, :, ct * 128:(ct + 1) * 128],
                                xT8_sb[:, k, :, :, :],
                                start=(k == 0),
                                stop=(k == 2),
                                perf_mode=mybir.MatmulPerfMode.DoubleRow,
                            )
                    for ct in cts:
                        qp = qps[ct]
                        if ct < 6:
                            for hb in range(2):
                                qbias = qbc_sb[:, 2 * g + hb, ct:ct + 1]
                                dst = qkT_sb[:, ct, hb * NF:(hb + 1) * NF]
                                srcp = qp[:, hb * NF:(hb + 1) * NF]
                                if ct % 2 == 0:
                                    nc.vector.tensor_scalar(
                                        out=dst, in0=srcp, scalar1=1.0 / QS,
                                        scalar2=qbias,
                                        op0=mybir.AluOpType.mult,
                                        op1=mybir.AluOpType.add,
                                    )
                                else:
                                    nc.scalar.activation(
                                        dst, srcp,
                                        mybir.ActivationFunctionType.Identity,
                                        bias=qbias, scale=1.0 / QS,
                                    )
                        else:
                            if ct % 2 == 0:
                                nc.vector.tensor_scalar_mul(
                                    qkT_sb[:, ct, :], qp[:], 1.0 / QS)
                            else:
                                nc.scalar.activation(
                                    qkT_sb[:, ct, :], qp[:],
                                    mybir.ActivationFunctionType.Identity,
                                    scale=1.0 / QS,
                                )

                for hb in range(2):
                    b = 2 * g + hb
                    is_last = (b == BPC - 1)

                    vpb_t = sb_vpb.tile([128, DIM], BF16, tag="vpb", name=f"vpb_{b}")
                    nc.gpsimd.dma_start(out=vpb_t[:], in_=vpb_d.ap()[b])

                    # ---- v (natural layout, 65-wide head slots, col 64 = 1s) ----
                    v_sb = sb_v.tile([128, 2, HEADS, 65], BF16, tag="v",
                                     name=f"v_{b}")
                    nc.vector.memset(v_sb[:, :, :, 64:65], 1.0)
                    for t, (off, mt) in enumerate(TOK_TILES):
                        vp = ps.tile([128, 512], F32, tag="ps", name=f"vp_{b}_{t}")
                        vp2 = ps.tile([128, 256], F32, tag="ps", name=f"vp2_{b}_{t}")
                        for k in range(KT):
                            xsl = xT_sb[:, hb, k, off:off + mt]
                            nc.tensor.matmul(
                                vp[0:mt, :], xsl, wt_sb[:, k, 0:512],
                                start=(k == 0), stop=(k == KT - 1),
                            )
                            nc.tensor.matmul(
                                vp2[0:mt, :], xsl, wt_sb[:, k, 512:768],
                                start=(k == 0), stop=(k == KT - 1),
                            )
                        # v_sb head axis is in PAIR_PERM order: even head h ->
                        # slot h//2, odd head h -> slot 6 + h//2
                        nc.vector.tensor_copy(
                            v_sb[0:mt, t, :, :].rearrange(
                                "p (par a) c -> p a par c", par=2)[:, 0:4, :, 0:64],
                            vp[0:mt, :].rearrange("p (a par d) -> p a par d",
                                                  par=2, d=HEAD_DIM),
                        )
                        nc.scalar.copy(
                            v_sb[0:mt, t, :, :].rearrange(
                                "p (par a) c -> p a par c", par=2)[:, 4:6, :, 0:64],
                            vp2[0:mt, :].rearrange("p (a par d) -> p a par d",
                                                   par=2, d=HEAD_DIM),
                        )

                    # ---- attention, software-pipelined by one head-pair ----
                    cT_sb = [sb_cT.tile([128, NF], BF16, tag="cT",
                                        name=f"cT_{b}_{j}") for j in range(6)]
                    norm_state = {}
                    exp_tiles = {}
                    self_st = make_pstate(b, cT_sb, vpb_t) if is_last else None

                    def emit_norm(sp, st=None, cT_=None):
                        # pb broadcast + cT multiply for pair sp (deferred one
                        # step so the PE never waits on the reciprocal chain)
                        stt = st if st is not None else norm_state
                        cc = cT_ if cT_ is not None else cT_sb
                        po_full, rec_sb, bb = stt.pop(sp)
                        pb = ps_pb.tile([128, NF], F32, tag="pb",
                                        name=f"pb_{bb}_{sp}")
                        nc.tensor.matmul(
                            pb[0:128, :], twohot_sb[64:66, 0:128],
                            rec_sb[64:66, 0:NF], start=True, stop=True,
                        )
                        nc.vector.tensor_mul(cc[sp][:], po_full[:], pb[:])

                    # flush the previous batch's last normalization now that
                    # the v-phase matmuls cover its reciprocal latency
                    if pend_norm[0] is not None:
                        pstt, pcT = pend_norm[0]
                        for psp in sorted(pstt.keys()):
                            emit_norm(psp, st=pstt, cT_=pcT)
                        pend_norm[0] = None

                    def emit_sc(sp):
                        h0, h1 = PAIRS[sp]
                        expT = sb_exp.tile([128, 2, NF2], BF16, tag="expT",
                                           name=f"expT_{b}_{sp}")
                        scs = []
                        for t, (off, mt) in enumerate(TOK_TILES):
                            sc = ps.tile([128, NF2], F32, tag="ps",
                                         name=f"sc_{b}_{sp}_{t}")
                            nc.tensor.matmul(
                                sc[0:mt, 0:NF], kT(qkT_sb, h0, hb, off, mt),
                                qT(qkT_sb, h0, hb), start=True, stop=True,
                            )
                            nc.tensor.matmul(
                                sc[0:mt, NF:NF2], kT(qkT_sb, h1, hb, off, mt),
                                qT(qkT_sb, h1, hb), start=True, stop=True,
                            )
                            scs.append(sc)
                        for t, (off, mt) in enumerate(TOK_TILES):
                            nc.scalar.activation(
                                expT[0:mt, t, :], scs[t][0:mt, :],
                                mybir.ActivationFunctionType.Exp,
                            )
                            eng = nc.vector if t == 0 else nc.gpsimd
                            eng.tensor_mul(
                                expT[0:mt, t, :], expT[0:mt, t, :],
                                relt_sb[0:mt, sp, t, :],
                            )
                        exp_tiles[sp] = expT

                    # prev-batch proj schedule: normal batches spread 6 chunks
                    # over the 6 steps; the last batch compresses them into
                    # steps 0-3 and starts its own projection early
                    if prev_proj[0] is not None:
                        prev_st = make_pstate(*prev_proj[0])
                        if is_last:
                            sched = {
                                0: [("pj", prev_st, 0, [0, 1, 2, 3])],
                                1: [("pj", prev_st, 0, [4, 5]),
                                    ("fin", prev_st, 0)],
                                2: [("pj", prev_st, 1, [0, 1, 2, 3])],
                                3: [("pj", prev_st, 1, [4, 5]),
                                    ("fin", prev_st, 1),
                                    ("pj", self_st, 0, [0])],
                                4: [("pj", self_st, 0, [1])],
                                5: [("pj", self_st, 0, [2, 3, 4])],
                            }
                        else:
                            sched = {
                                0: [("pj", prev_st, 0, [0, 1])],
                                1: [("pj", prev_st, 0, [2, 3])],
                                2: [("pj", prev_st, 0, [4, 5]),
                                    ("fin", prev_st, 0)],
                                3: [("pj", prev_st, 1, [0, 1])],
                                4: [("pj", prev_st, 1, [2, 3])],
                                5: [("pj", prev_st, 1, [4, 5]),
                                    ("fin", prev_st, 1)],
                            }
                    else:
                        sched = {}

                    emit_sc(0)
                    for sp in range(6):
                        if sp < 5:
                            emit_sc(sp + 1)
                        if sp >= 2 and (sp - 2) in norm_state:
                            emit_norm(sp - 2)
                        if is_last and sp == 5 and 4 in norm_state:
                            emit_norm(4)
                        for item in sched.get(sp, []):
                            if item[0] == "pj":
                                emit_pj(item[1], item[2], item[3])
                            else:
                                emit_pj_fin(item[1], item[2])
                        # one PSUM bank per head: a start=True resets the
                        # whole bank's has-written bits, so accumulation
                        # groups must not interleave within a bank
                        expT = exp_tiles.pop(sp)
                        pos = [ps.tile([65, NF], F32, tag="ps",
                                       name=f"po_{b}_{sp}_{i}") for i in (0, 1)]
                        for i in (0, 1):
                            for t, (off, mt) in enumerate(TOK_TILES):
                                nc.tensor.matmul(
                                    pos[i][0:65, :],
                                    v_sb[0:mt, t, 2 * sp + i, :],
                                    expT[0:mt, t, i * NF:(i + 1) * NF],
                                    start=(t == 0), stop=(t == 1),
                                )
                        # row 64 of po = per-token exp sums (65-col
                        # stationary with a trailing ones column)
                        po_sb = sb_po.tile([66, NF2], F32, tag="po",
                                           name=f"po_sb_{b}_{sp}")
                        nc.vector.tensor_copy(po_sb[0:65, 0:NF], pos[0][0:65, :])
                        nc.scalar.copy(po_sb[0:65, NF:NF2], pos[1][0:65, :])
                        # h1 sums shift to partition 65 so one K=2 two-hot
                        # matmul broadcasts both heads' reciprocals at once
                        nc.sync.dma_start(out=po_sb[65:66, 0:NF],
                                          in_=po_sb[64:65, NF:NF2])
                        # rows 0:64 are po values (garbage out, never read);
                        # the custom DVE op needs base partition 0
                        rec_f = sb_rec.tile([66, NF], F32, tag="recf",
                                            name=f"recf_{b}_{sp}")
                        nc.vector.reciprocal_approx_fast(out=rec_f[0:66, :],
                                                         in_=po_sb[0:66, 0:NF])
                        rec_sb = sb_rec.tile([66, NF], BF16, tag="rec",
                                             name=f"rec_{b}_{sp}")
                        nc.vector.tensor_copy(rec_sb[64:66, :], rec_f[64:66, :])
                        # assemble both heads' po on partitions 0:128 so the
                        # cT normalization is a single 128-partition multiply
                        po_full = sb_pohi.tile([128, NF], F32, tag="pohi",
                                               name=f"pofull_{b}_{sp}")
                        nc.sync.dma_start(out=po_full[0:64, :],
                                          in_=po_sb[0:64, 0:NF])
                        nc.sync.dma_start(out=po_full[64:128, :],
                                          in_=po_sb[0:64, NF:NF2])
                        norm_state[sp] = (po_full, rec_sb, b)

                    if is_last:
                        emit_norm(5)
                        emit_pj(self_st, 0, [5])
                        emit_pj_fin(self_st, 0)
                        emit_pj(self_st, 1, [0, 1, 2, 3, 4, 5])
                        emit_pj_fin(self_st, 1)
                    else:
                        pend_norm[0] = (norm_state, cT_sb)
                        prev_proj[0] = (b, cT_sb, vpb_t)

    nc.compile()
    nc.m = get_hw_module(nc.m)
    return nc


def _host_prep(x, qkv_weight, q_bias, v_bias, rel_table, proj_weight, proj_bias,
               b_idx, rel_index):
    x = np.asarray(x, dtype=np.float32)
    # xt8[b, k, p, n] = x[b, n, 128k+p]
    xt = np.ascontiguousarray(
        x.transpose(0, 2, 1).reshape(B, KT, 128, N)
        .transpose(0, 2, 1, 3)).astype(BFNP)
    # pair-interleaved fp8 x: [pair, p, k, ko, hb, n]
    xc8 = (x.transpose(0, 2, 1).reshape(B // 2, 2, 3, 2, 128, N)
           .transpose(0, 4, 2, 3, 1, 5)).astype(FP8NP)
    xc8 = np.ascontiguousarray(xc8)
    W = np.asarray(qkv_weight, dtype=np.float32).copy()
    W[:DIM] *= np.float32(SCALE)
    # v-only bf16 weights
    wt = np.ascontiguousarray(W[2 * DIM:].T.reshape(KT, 128, DIM)).astype(BFNP)
    # q,k weights in fp8, pre-scaled by QS; c = 256k + 128ko + p
    wqk = (W[0:2 * DIM] * np.float32(QS)).T  # [768c, 1536m]
    wt8 = np.ascontiguousarray(
        wqk.reshape(3, 2, 128, 2 * DIM).transpose(0, 2, 1, 3)).astype(FP8NP)
    pwtT = np.asarray(proj_weight, dtype=np.float32).T  # [c', o]
    pwtT = pwtT.reshape(HEADS, HEAD_DIM, DIM)[PAIR_PERM].reshape(DIM, DIM)
    pwt = np.ascontiguousarray(
        pwtT.reshape(KT, 128, DIM).transpose(1, 0, 2)).astype(BFNP)

    bi = np.asarray(b_idx).astype(np.int64)
    qb_all = (np.asarray(q_bias, dtype=np.float32)[bi] * np.float32(SCALE))
    vb_all = np.asarray(v_bias, dtype=np.float32)[bi]
    # softmax rows sum to 1, so attn @ (1 x vb) == 1 x vb; push the v bias
    # through the projection into the proj bias
    pb_all = (np.asarray(proj_bias, dtype=np.float32)[bi]
              + vb_all @ np.asarray(proj_weight, dtype=np.float32).T).astype(BFNP)
    pb_bcast = np.ascontiguousarray(
        np.broadcast_to(pb_all[:, None, :], (B, 128, DIM)))

    ridx = np.asarray(rel_index).astype(np.int64)
    rel = np.asarray(rel_table, dtype=np.float32)[ridx.reshape(-1)]
    rel = rel.reshape(N, N, HEADS)  # [n, m, h]
    relth = np.zeros((HEADS, 2, 128, NF), dtype=np.float32)
    for t, (off, mt) in enumerate(TOK_TILES):
        # relth[h, t, p, n] = exp(rel[n, off+p, h])
        relth[:, t, 0:mt, :] = np.exp(rel[:, off:off + mt, :].transpose(2, 1, 0))
    # pair-merged: relt[sp, t, p, i*NF+n] = relth[PAIRS[sp][i], t, p, n]
    relt = np.ascontiguousarray(
        relth[PAIR_PERM].reshape(6, 2, 2, 128, NF)
        .transpose(0, 2, 3, 1, 4).reshape(6, 2, 128, NF2)
        .transpose(2, 0, 1, 3)).astype(BFNP)

    twohot = np.zeros((128, 128), dtype=BFNP)
    twohot[64, 0:64] = 1.0
    twohot[65, 64:128] = 1.0

    in_maps = []
    for c in range(NCORES):
        sl = slice(c * BPC, (c + 1) * BPC)
        qbc = np.ascontiguousarray(
            qb_all[sl].reshape(BPC, KT, 128).transpose(2, 0, 1))
        vpb = np.ascontiguousarray(pb_bcast[sl])
        in_maps.append({
            "xt8": np.ascontiguousarray(xt[sl]),
            "xt8f8": np.ascontiguousarray(xc8[c * (BPC // 2):(c + 1) * (BPC // 2)]),
            "wt8": wt8,
            "wt": wt,
            "pwt": pwt,
            "qbc": qbc,
            "vpb8": vpb,
            "relt": relt,
            "twohot": twohot,
        })
    return in_maps


def _install_ntff_hook():
    """Provide antenv.axon_hooks (absent from this image) so bass_utils can
    capture NTFF profiles through libaxon_pjrt.so, and keep artifacts local."""
    if _CACHE.get("hook_installed"):
        return
    import sys
    import types
    import ctypes
    import contextlib

    so_path = "/opt/axon/libaxon_pjrt.so"
    lib = ctypes.CDLL(so_path)
    lib.axon_start_nrt_profile.argtypes = [
        ctypes.POINTER(ctypes.c_int64),
        ctypes.c_size_t,
    ]
    lib.axon_start_nrt_profile.restype = ctypes.c_int64
    lib.axon_stop_nrt_profile.argtypes = [ctypes.c_char_p]
    lib.axon_stop_nrt_profile.restype = ctypes.c_int64

    @contextlib.contextmanager
    def _hook(output_dir, device_ids):
        import jax

        jax.devices()
        if device_ids:
            ids = (ctypes.c_int64 * len(device_ids))(*device_ids)
            rc = lib.axon_start_nrt_profile(ids, len(device_ids))
        else:
            rc = lib.axon_start_nrt_profile(None, 0)
        if rc != 0:
            raise RuntimeError(f"axon_start_nrt_profile rc={rc}")
        try:
            yield
        finally:
            n = lib.axon_stop_nrt_profile(str(output_dir).encode())
            print(f"ntff profile: {n} file(s) written to {output_dir}")

    mod = types.ModuleType("antenv.axon_hooks")
    mod.get_axon_ntff_profile_hook = lambda: _hook
    mod.set_axon_ntff_profile_hook = lambda h: None
    sys.modules["antenv.axon_hooks"] = mod

    import concourse.bass_utils as bu

    bu.upload_artifacts = lambda tmpdir: str(tmpdir)
    _CACHE["hook_installed"] = True


def kernel(**inputs):
    if "nc" not in _CACHE:
        _CACHE["nc"] = _build_module()
    nc = _CACHE["nc"]

    in_maps = _host_prep(**inputs)
    trace = os.environ.get("KERNEL_TRACE", "0") == "1"
    tmpdir = None
    if trace:
        _install_ntff_hook()
        tmpdir = os.environ.get("KERNEL_TRACE_DIR") or None
    res = run_bass_kernel_spmd(nc, in_maps, core_ids=list(range(NCORES)), trace=trace,
                               tmpdir=tmpdir)
    if trace:
        _CACHE["last_exec_time_ns"] = res.exec_time_ns
        _CACHE["last_results"] = res

    y = np.concatenate([res.results[c]["y8"] for c in range(NCORES)], axis=0)
    return y


# revision 29
# speedup vs baseline: 1.0079x; 1.0079x over previous
"""Trainium2 Bass kernel for BEiT attention block (nn_Beit_9560597201107).

Data-parallel over batch: 64 batches -> 8 NeuronCores x 8 batches each.
Fully transposed dataflow (channels on partitions) so the softmax'd
attention matrix is never transposed on-chip:

  xT = x.T (host)                                  [768, 197]
  qkT[c, n] = sum_k WT[k, c] xT[k, n] + bias       [1536, 197]  (q pre-scaled)
  v[m, d]   = sum_k xT[k, m] WT_v[k, d] + bias     [197, 768]   (natural)
  scT[m, n] = sum_d kT[d, m] qT[d, n]              per head
  eT = exp(scT) * exp_rel_T                        (rel bias via exp-mult)
  po[d, n], sums[n] = sum_m [v|1][m, d] eT[m, n]   (ones col -> row 64 = sums)
  cT = po * broadcast(1/sums)   (PE ones-outer-product broadcast)
  y[n, o] = sum_c cT[c, n] projWT[c, o] + bias

Matmuls run in bfloat16 (fp32 PSUM accumulation, 1 cycle/row at any free
size), except the big qk projection which uses fp8e4 with DoubleRow perf
mode (0.5 cycles/row, half the PE energy -> less power throttling).  fp8
q/k weights are pre-scaled x64 on the host to stay out of the subnormal
range and rescaled by 1/64 in the PSUM->SBUF copy.  The attention
normalization is deferred two head-pair steps and the scores of the next
pair are prefetched so the PE never waits on the exp/reciprocal chains.
"""

import os
import numpy as np
import ml_dtypes

import concourse.bass as bass
import concourse.bacc as bacc
import concourse.mybir as mybir
import concourse.tile as tile
from concourse.bass_utils import run_bass_kernel_spmd
from concourse.bass_interp import get_hw_module

B, N, DIM, HEADS, NBS = 64, 197, 768, 12, 10
HEAD_DIM = DIM // HEADS
SCALE = HEAD_DIM ** -0.5
NCORES = 8
BPC = B // NCORES          # batches per core
KT = DIM // 128            # 6 contraction tiles
NF = N                     # token free-dim, exact (bf16 full rate at any size)
NF2 = 2 * NF
TOK_TILES = [(0, 128), (128, 69)]  # (offset, size) over the 197 tokens
# Heads grouped in same-parity pairs: both heads of a pair live at the same
# 64-partition half of qkT, so their back-to-back matmuls into one PSUM bank
# use the same PE row group.
PAIRS = [(0, 2), (4, 6), (8, 10), (1, 3), (5, 7), (9, 11)]
PAIR_PERM = [h for p in PAIRS for h in p]

F32 = mybir.dt.float32
FP8 = mybir.dt.float8e4
FP8NP = ml_dtypes.float8_e4m3
QS = 64.0  # fp8 weight pre-scale (keeps q/k weights out of subnormal range)
BF16 = mybir.dt.bfloat16
BFNP = ml_dtypes.bfloat16

_CACHE = {}


def _build_module():
    nc = bacc.Bacc("TRN2", target_bir_lowering=False, debug=False)

    # host-transposed x: xt8[b, k, p, n] = x[b, n, 128k+p]
    xt8_d = nc.dram_tensor("xt8", [BPC, 128, KT, NF], BF16, kind="ExternalInput")
    wt_d = nc.dram_tensor("wt", [KT, 128, DIM], BF16, kind="ExternalInput")
    wt8_d = nc.dram_tensor("wt8", [3, 128, 2, 2 * DIM], FP8, kind="ExternalInput")
    xt8f8_d = nc.dram_tensor("xt8f8", [BPC // 2, 128, 3, 2, 2, NF], FP8,
                             kind="ExternalInput")
    pwt_d = nc.dram_tensor("pwt", [128, KT, DIM], BF16, kind="ExternalInput")
    qbc_d = nc.dram_tensor("qbc", [128, BPC, KT], F32, kind="ExternalInput")
    vpb_d = nc.dram_tensor("vpb8", [BPC, 128, DIM], BF16, kind="ExternalInput")
    relt_d = nc.dram_tensor("relt", [128, 6, 2, NF2], BF16, kind="ExternalInput")
    twohot_d = nc.dram_tensor("twohot", [128, 128], BF16, kind="ExternalInput")
    y8_d = nc.dram_tensor("y8", [BPC, N, DIM], F32, kind="ExternalOutput")

    with tile.TileContext(nc) as tc:
        with (
            tc.tile_pool(name="const", bufs=1) as constp,
            tc.tile_pool(name="sb_xT", bufs=4) as sb_xT,
            tc.tile_pool(name="sb_qkT", bufs=2) as sb_qkT,
            tc.tile_pool(name="sb_v", bufs=2) as sb_v,
            tc.tile_pool(name="sb_exp", bufs=2) as sb_exp,
            tc.tile_pool(name="sb_po", bufs=3) as sb_po,
            tc.tile_pool(name="sb_pohi", bufs=3) as sb_pohi,
            tc.tile_pool(name="sb_rec", bufs=6) as sb_rec,
            tc.tile_pool(name="sb_cT", bufs=12) as sb_cT,
            tc.tile_pool(name="sb_out", bufs=2) as sb_out,
            tc.tile_pool(name="sb_vpb", bufs=2) as sb_vpb,
            tc.tile_pool(name="ps", bufs=5, space="PSUM") as ps,
            tc.tile_pool(name="ps_pj", bufs=2, space="PSUM") as ps_pj,
            tc.tile_pool(name="ps_pb", bufs=1, space="PSUM") as ps_pb,
        ):
            # ---- persistent data; wt chunked per k-tile so the first qkT
            # ---- matmuls can start as soon as chunk 0 lands
            qbc_sb = constp.tile([128, BPC, KT], F32)
            nc.sync.dma_start(out=qbc_sb[:], in_=qbc_d.ap())
            wt8_sb = constp.tile([128, 3, 2, 2 * DIM], FP8)
            nc.gpsimd.dma_start(out=wt8_sb[:, 0, :, 0:512],
                                in_=wt8_d.ap()[0][:, :, 0:512])
            nc.gpsimd.dma_start(out=wt8_sb[:, 0, :, 512:2 * DIM],
                                in_=wt8_d.ap()[0][:, :, 512:2 * DIM])
            for k in range(1, 3):
                nc.gpsimd.dma_start(out=wt8_sb[:, k, :, :], in_=wt8_d.ap()[k])
            wt_sb = constp.tile([128, KT, DIM], BF16)
            for k in range(KT):
                nc.gpsimd.dma_start(out=wt_sb[:, k, :], in_=wt_d.ap()[k])
            relt_sb = constp.tile([128, 6, 2, NF2], BF16)
            nc.gpsimd.dma_start(out=relt_sb[:], in_=relt_d.ap())
            pwt_sb = constp.tile([128, KT, DIM], BF16)
            nc.gpsimd.dma_start(out=pwt_sb[:], in_=pwt_d.ap())
            twohot_sb = constp.tile([128, 128], BF16)
            nc.gpsimd.dma_start(out=twohot_sb[:], in_=twohot_d.ap())

            def kT(qkT_sb, h, hb, off, mt):
                base = (h % 2) * 64
                return qkT_sb[base:base + 64, 6 + h // 2,
                              hb * NF + off:hb * NF + off + mt]

            def qT(qkT_sb, h, hb):
                base = (h % 2) * 64
                return qkT_sb[base:base + 64, h // 2, hb * NF:(hb + 1) * NF]

            # ---- projection machinery (generic over prev/self batch) ----
            def make_pstate(b_, cT_, vpb_):
                return {"b": b_, "cT": cT_, "vpb": vpb_, "ps": {}}

            def emit_pj(st, t, js):
                off, mt = TOK_TILES[t]
                if t not in st["ps"]:
                    st["ps"][t] = (
                        ps_pj.tile([128, 512], F32, tag="pj",
                                   name=f"pr_{st['b']}_{t}"),
                        ps_pj.tile([128, 256], F32, tag="pj",
                                   name=f"pr2_{st['b']}_{t}"),
                    )
                pr, pr2 = st["ps"][t]
                for j in js:
                    nc.tensor.matmul(
                        pr[0:mt, :], st["cT"][j][:, off:off + mt],
                        pwt_sb[:, j, 0:512], start=(j == 0), stop=(j == 5),
                    )
                    nc.tensor.matmul(
                        pr2[0:mt, :], st["cT"][j][:, off:off + mt],
                        pwt_sb[:, j, 512:768], start=(j == 0), stop=(j == 5),
                    )

            def emit_pj_fin(st, t):
                off, mt = TOK_TILES[t]
                pr, pr2 = st["ps"][t]
                out_sb = sb_out.tile([128, DIM], F32, tag="out",
                                     name=f"out_{st['b']}_{t}")
                nc.vector.tensor_add(out_sb[0:mt, 0:512], pr[0:mt, :],
                                     st["vpb"][0:mt, 0:512])
                nc.vector.tensor_add(out_sb[0:mt, 512:768], pr2[0:mt, :],
                                     st["vpb"][0:mt, 512:768])
                nc.sync.dma_start(out=y8_d.ap()[st["b"], off:off + mt, :],
                                  in_=out_sb[0:mt, :])

            prev_proj = [None]
            pend_norm = [None]

            for g in range(BPC // 2):
                # ---- load host-transposed x for the batch pair (sync queue
                # ---- so it runs in parallel with the const loads) ----
                xT8_sb = sb_xT.tile([128, 3, 2, 2, NF], FP8, tag="xT8",
                                    name=f"xT8_{g}")
                for kk in range(3):
                    nc.sync.dma_start(out=xT8_sb[:, kk, :, :, :],
                                      in_=xt8f8_d.ap()[g][:, kk])
                xT_sb = sb_xT.tile([128, 2, KT, NF], BF16, tag="xT", name=f"xT_{g}")
                for hb in range(2):
                    nc.sync.dma_start(
                        out=xT_sb[:, hb, :, :],
                        in_=xt8_d.ap()[2 * g + hb],
                    )

                # ---- qkT for both batches; k-outer in two ct-halves so the
                # ---- first matmuls only need wt chunk 0 ----
                qkT_sb = sb_qkT.tile([128, 12, NF2], BF16, tag="qkT", name=f"qkT_{g}")
                for third in range(3):
                    cts = list(range(4 * third, 4 * third + 4))
                    qps = {ct: ps.tile([128, NF2], F32, tag="ps",
                                       name=f"qp_{g}_{ct}") for ct in cts}
                    for k in range(3):
                        for ct in cts:
                            nc.tensor.matmul(
                                qps[ct][:],
                                wt8_sb[:, k, :, ct * 128:(ct + 1) * 128],
                                xT8_sb[:, k, :, :, :],
                                start=(k == 0),
                                stop=(k == 2),
                                perf_mode=mybir.MatmulPerfMode.DoubleRow,
                            )
                    for ct in cts:
                        qp = qps[ct]
                        if ct < 6:
                            for hb in range(2):
                                qbias = qbc_sb[:, 2 * g + hb, ct:ct + 1]
                                dst = qkT_sb[:, ct, hb * NF:(hb + 1) * NF]
                                srcp = qp[:, hb * NF:(hb + 1) * NF]
                                if ct % 2 == 0:
                                    nc.vector.tensor_scalar(
                                        out=dst, in0=srcp, scalar1=1.0 / QS,
                                        scalar2=qbias,
                                        op0=mybir.AluOpType.mult,
                                        op1=mybir.AluOpType.add,
                                    )
                                else:
                                    nc.scalar.activation(
                                        dst, srcp,
                                        mybir.ActivationFunctionType.Identity,
                                        bias=qbias, scale=1.0 / QS,
                                    )
                        else:
                            if ct % 2 == 0:
                                nc.vector.tensor_scalar_mul(
                                    qkT_sb[:, ct, :], qp[:], 1.0 / QS)
                            else:
                                nc.scalar.activation(
                                    qkT_sb[:, ct, :], qp[:],
                                    mybir.ActivationFunctionType.Identity,
                                    scale=1.0 / QS,
                                )

                for hb in range(2):
                    b = 2 * g + hb
                    is_last = (b == BPC - 1)

                    vpb_t = sb_vpb.tile([128, DIM], BF16, tag="vpb", name=f"vpb_{b}")
                    nc.gpsimd.dma_start(out=vpb_t[:], in_=vpb_d.ap()[b])

                    # ---- v (natural layout, 65-wide head slots, col 64 = 1s) ----
                    v_sb = sb_v.tile([128, 2, HEADS, 65], BF16, tag="v",
                                     name=f"v_{b}")
                    nc.vector.memset(v_sb[:, :, :, 64:65], 1.0)
                    for t, (off, mt) in enumerate(TOK_TILES):
                        vp = ps.tile([128, 512], F32, tag="ps", name=f"vp_{b}_{t}")
                        vp2 = ps.tile([128, 256], F32, tag="ps", name=f"vp2_{b}_{t}")
                        for k in range(KT):
                            xsl = xT_sb[:, hb, k, off:off + mt]
                            nc.tensor.matmul(
                                vp[0:mt, :], xsl, wt_sb[:, k, 0:512],
                                start=(k == 0), stop=(k == KT - 1),
                            )
                            nc.tensor.matmul(
                                vp2[0:mt, :], xsl, wt_sb[:, k, 512:768],
                                start=(k == 0), stop=(k == KT - 1),
                            )
                        # v_sb head axis is in PAIR_PERM order: even head h ->
                        # slot h//2, odd head h -> slot 6 + h//2
                        nc.vector.tensor_copy(
                            v_sb[0:mt, t, :, :].rearrange(
                                "p (par a) c -> p a par c", par=2)[:, 0:4, :, 0:64],
                            vp[0:mt, :].rearrange("p (a par d) -> p a par d",
                                                  par=2, d=HEAD_DIM),
                        )
                        nc.scalar.copy(
                            v_sb[0:mt, t, :, :].rearrange(
                                "p (par a) c -> p a par c", par=2)[:, 4:6, :, 0:64],
                            vp2[0:mt, :].rearrange("p (a par d) -> p a par d",
                                                   par=2, d=HEAD_DIM),
                        )

                    # ---- attention, software-pipelined by one head-pair ----
                    cT_sb = [sb_cT.tile([128, NF], BF16, tag="cT",
                                        name=f"cT_{b}_{j}") for j in range(6)]
                    norm_state = {}
                    exp_tiles = {}
                    self_st = make_pstate(b, cT_sb, vpb_t) if is_last else None

                    def emit_norm(sp, st=None, cT_=None):
                        # pb broadcast + cT multiply for pair sp (deferred one
                        # step so the PE never waits on the reciprocal chain)
                        stt = st if st is not None else norm_state
                        cc = cT_ if cT_ is not None else cT_sb
                        po_full, rec_sb, bb = stt.pop(sp)
                        pb = ps_pb.tile([128, NF], F32, tag="pb",
                                        name=f"pb_{bb}_{sp}")
                        nc.tensor.matmul(
                            pb[0:128, :], twohot_sb[64:66, 0:128],
                            rec_sb[64:66, 0:NF], start=True, stop=True,
                        )
                        nc.vector.tensor_mul(cc[sp][:], po_full[:], pb[:])

                    # flush the previous batch's last normalization now that
                    # the v-phase matmuls cover its reciprocal latency
                    if pend_norm[0] is not None:
                        pstt, pcT = pend_norm[0]
                        for psp in sorted(pstt.keys()):
                            emit_norm(psp, st=pstt, cT_=pcT)
                        pend_norm[0] = None

                    def emit_sc(sp):
                        h0, h1 = PAIRS[sp]
                        expT = sb_exp.tile([128, 2, NF2], BF16, tag="expT",
                                           name=f"expT_{b}_{sp}")
                        scs = []
                        for t, (off, mt) in enumerate(TOK_TILES):
                            sc = ps.tile([128, NF2], F32, tag="ps",
                                         name=f"sc_{b}_{sp}_{t}")
                            nc.tensor.matmul(
                                sc[0:mt, 0:NF], kT(qkT_sb, h0, hb, off, mt),
                                qT(qkT_sb, h0, hb), start=True, stop=True,
                            )
                            nc.tensor.matmul(
                                sc[0:mt, NF:NF2], kT(qkT_sb, h1, hb, off, mt),
                                qT(qkT_sb, h1, hb), start=True, stop=True,
                            )
                            scs.append(sc)
                        for t, (off, mt) in enumerate(TOK_TILES):
                            nc.scalar.activation(
                                expT[0:mt, t, :], scs[t][0:mt, :],
                                mybir.ActivationFunctionType.Exp,
                            )
                            eng = nc.vector if t == 0 else nc.gpsimd
                            eng.tensor_mul(
                                expT[0:mt, t, :], expT[0:mt, t, :],
                                relt_sb[0:mt, sp, t, :],
                            )
                        exp_tiles[sp] = expT

                    # prev-batch proj schedule: normal batches spread 6 chunks
                    # over the 6 steps; the last batch compresses them into
                    # steps 0-3 and starts its own projection early
                    if prev_proj[0] is not None:
                        prev_st = make_pstate(*prev_proj[0])
                        if is_last:
                            sched = {
                                0: [("pj", prev_st, 0, [0, 1, 2, 3])],
                                1: [("pj", prev_st, 0, [4, 5]),
                                    ("fin", prev_st, 0)],
                                2: [("pj", prev_st, 1, [0, 1, 2, 3])],
                                3: [("pj", prev_st, 1, [4, 5]),
                                    ("fin", prev_st, 1),
                                    ("pj", self_st, 0, [0])],
                                4: [("pj", self_st, 0, [1])],
                                5: [("pj", self_st, 0, [2, 3, 4])],
                            }
                        else:
                            sched = {
                                0: [("pj", prev_st, 0, [0, 1])],
                                1: [("pj", prev_st, 0, [2, 3])],
                                2: [("pj", prev_st, 0, [4, 5]),
                                    ("fin", prev_st, 0)],
                                3: [("pj", prev_st, 1, [0, 1])],
                                4: [("pj", prev_st, 1, [2, 3])],
                                5: [("pj", prev_st, 1, [4, 5]),
                                    ("fin", prev_st, 1)],
                            }
                    else:
                        sched = {}

                    emit_sc(0)
                    for sp in range(6):
                        if sp < 5:
                            emit_sc(sp + 1)
                        if sp >= 2 and (sp - 2) in norm_state:
                            emit_norm(sp - 2)
                        if is_last and sp == 5 and 4 in norm_state:
                            emit_norm(4)
                        for item in sched.get(sp, []):
                            if item[0] == "pj":
                                emit_pj(item[1], item[2], item[3])
                            else:
                                emit_pj_fin(item[1], item[2])
                        # one PSUM bank per head: a start=True resets the
                        # whole bank's has-written bits, so accumulation
                        # groups must not interleave within a bank
                        expT = exp_tiles.pop(sp)
                        pos = [ps.tile([65, NF], F32, tag="ps",
                                       name=f"po_{b}_{sp}_{i}") for i in (0, 1)]
                        for i in (0, 1):
                            for t, (off, mt) in enumerate(TOK_TILES):
                                nc.tensor.matmul(
                                    pos[i][0:65, :],
                                    v_sb[0:mt, t, 2 * sp + i, :],
                                    expT[0:mt, t, i * NF:(i + 1) * NF],
                                    start=(t == 0), stop=(t == 1),
                                )
                        # row 64 of po = per-token exp sums (65-col
                        # stationary with a trailing ones column)
                        po_sb = sb_po.tile([66, NF2], F32, tag="po",
                                           name=f"po_sb_{b}_{sp}")
                        nc.vector.tensor_copy(po_sb[0:65, 0:NF], pos[0][0:65, :])
                        nc.scalar.copy(po_sb[0:65, NF:NF2], pos[1][0:65, :])
                        # h1 sums shift to partition 65 so one K=2 two-hot
                        # matmul broadcasts both heads' reciprocals at once
                        nc.sync.dma_start(out=po_sb[65:66, 0:NF],
                                          in_=po_sb[64:65, NF:NF2])
                        # rows 0:64 are po values (garbage out, never read);
                        # the custom DVE op needs base partition 0
                        rec_f = sb_rec.tile([66, NF], F32, tag="recf",
                                            name=f"recf_{b}_{sp}")
                        nc.vector.reciprocal_approx_fast(out=rec_f[0:66, :],
                                                         in_=po_sb[0:66, 0:NF])
                        rec_sb = sb_rec.tile([66, NF], BF16, tag="rec",
                                             name=f"rec_{b}_{sp}")
                        nc.vector.tensor_copy(rec_sb[64:66, :], rec_f[64:66, :])
                        # assemble both heads' po on partitions 0:128 so the
                        # cT normalization is a single 128-partition multiply
                        po_full = sb_pohi.tile([128, NF], F32, tag="pohi",
                                               name=f"pofull_{b}_{sp}")
                        nc.sync.dma_start(out=po_full[0:64, :],
                                          in_=po_sb[0:64, 0:NF])
                        nc.sync.dma_start(out=po_full[64:128, :],
                                          in_=po_sb[0:64, NF:NF2])
                        norm_state[sp] = (po_full, rec_sb, b)

                    if is_last:
                        emit_norm(5)
                        emit_pj(self_st, 0, [5])
                        emit_pj_fin(self_st, 0)
                        emit_pj(self_st, 1, [0, 1, 2, 3, 4, 5])
                        emit_pj_fin(self_st, 1)
                    else:
                        pend_norm[0] = (norm_state, cT_sb)
                        prev_proj[0] = (b, cT_sb, vpb_t)

    nc.compile()
    nc.m = get_hw_module(nc.m)
    return nc


def _host_prep(x, qkv_weight, q_bias, v_bias, rel_table, proj_weight, proj_bias,
               b_idx, rel_index):
    x = np.asarray(x, dtype=np.float32)
    # xt8[b, k, p, n] = x[b, n, 128k+p]
    xt = np.ascontiguousarray(
        x.transpose(0, 2, 1).reshape(B, KT, 128, N)
        .transpose(0, 2, 1, 3)).astype(BFNP)
    # pair-interleaved fp8 x: [pair, p, k, ko, hb, n]
    xc8 = (x.transpose(0, 2, 1).reshape(B // 2, 2, 3, 2, 128, N)
           .transpose(0, 4, 2, 3, 1, 5)).astype(FP8NP)
    xc8 = np.ascontiguousarray(xc8)
    W = np.asarray(qkv_weight, dtype=np.float32).copy()
    W[:DIM] *= np.float32(SCALE)
    # v-only bf16 weights
    wt = np.ascontiguousarray(W[2 * DIM:].T.reshape(KT, 128, DIM)).astype(BFNP)
    # q,k weights in fp8, pre-scaled by QS; c = 256k + 128ko + p
    wqk = (W[0:2 * DIM] * np.float32(QS)).T  # [768c, 1536m]
    wt8 = np.ascontiguousarray(
        wqk.reshape(3, 2, 128, 2 * DIM).transpose(0, 2, 1, 3)).astype(FP8NP)
    pwtT = np.asarray(proj_weight, dtype=np.float32).T  # [c', o]
    pwtT = pwtT.reshape(HEADS, HEAD_DIM, DIM)[PAIR_PERM].reshape(DIM, DIM)
    pwt = np.ascontiguousarray(
        pwtT.reshape(KT, 128, DIM).transpose(1, 0, 2)).astype(BFNP)

    bi = np.asarray(b_idx).astype(np.int64)
    qb_all = (np.asarray(q_bias, dtype=np.float32)[bi] * np.float32(SCALE))
    vb_all = np.asarray(v_bias, dtype=np.float32)[bi]
    # softmax rows sum to 1, so attn @ (1 x vb) == 1 x vb; push the v bias
    # through the projection into the proj bias
    pb_all = (np.asarray(proj_bias, dtype=np.float32)[bi]
              + vb_all @ np.asarray(proj_weight, dtype=np.float32).T).astype(BFNP)
    pb_bcast = np.ascontiguousarray(
        np.broadcast_to(pb_all[:, None, :], (B, 128, DIM)))

    ridx = np.asarray(rel_index).astype(np.int64)
    rel = np.asarray(rel_table, dtype=np.float32)[ridx.reshape(-1)]
    rel = rel.reshape(N, N, HEADS)  # [n, m, h]
    relth = np.zeros((HEADS, 2, 128, NF), dtype=np.float32)
    for t, (off, mt) in enumerate(TOK_TILES):
        # relth[h, t, p, n] = exp(rel[n, off+p, h])
        relth[:, t, 0:mt, :] = np.exp(rel[:, off:off + mt, :].transpose(2, 1, 0))
    # pair-merged: relt[sp, t, p, i*NF+n] = relth[PAIRS[sp][i], t, p, n]
    relt = np.ascontiguousarray(
        relth[PAIR_PERM].reshape(6, 2, 2, 128, NF)
        .transpose(0, 2, 3, 1, 4).reshape(6, 2, 128, NF2)
        .transpose(2, 0, 1, 3)).astype(BFNP)

    twohot = np.zeros((128, 128), dtype=BFNP)
    twohot[64, 0:64] = 1.0
    twohot[65, 64:128] = 1.0

    in_maps = []
    for c in range(NCORES):
        sl = slice(c * BPC, (c + 1) * BPC)
        qbc = np.ascontiguousarray(
            qb_all[sl].reshape(BPC, KT, 128).transpose(2, 0, 1))
        vpb = np.ascontiguousarray(pb_bcast[sl])
        in_maps.append({
            "xt8": np.ascontiguousarray(xt[sl]),
            "xt8f8": np.ascontiguousarray(xc8[c * (BPC // 2):(c + 1) * (BPC // 2)]),
            "wt8": wt8,
            "wt": wt,
            "pwt": pwt,
            "qbc": qbc,
            "vpb8": vpb,
            "relt": relt,
            "twohot": twohot,
        })
    return in_maps


def _install_ntff_hook():
    """Provide antenv.axon_hooks (absent from this image) so bass_utils can
    capture NTFF profiles through libaxon_pjrt.so, and keep artifacts local."""
    if _CACHE.get("hook_installed"):
        return
    import sys
    import types
    import ctypes
    import contextlib

    so_path = "/opt/axon/libaxon_pjrt.so"
    lib = ctypes.CDLL(so_path)
    lib.axon_start_nrt_profile.argtypes = [
        ctypes.POINTER(ctypes.c_int64),
        ctypes.c_size_t,
    ]
    lib.axon_start_nrt_profile.restype = ctypes.c_int64
    lib.axon_stop_nrt_profile.argtypes = [ctypes.c_char_p]
    lib.axon_stop_nrt_profile.restype = ctypes.c_int64

    @contextlib.contextmanager
    def _hook(output_dir, device_ids):
        import jax

        jax.devices()
        if device_ids:
            ids = (ctypes.c_int64 * len(device_ids))(*device_ids)
            rc = lib.axon_start_nrt_profile(ids, len(device_ids))
        else:
            rc = lib.axon_start_nrt_profile(None, 0)
        if rc != 0:
            raise RuntimeError(f"axon_start_nrt_profile rc={rc}")
        try:
            yield
        finally:
            n = lib.axon_stop_nrt_profile(str(output_dir).encode())
            print(f"ntff profile: {n} file(s) written to {output_dir}")

    mod = types.ModuleType("antenv.axon_hooks")
    mod.get_axon_ntff_profile_hook = lambda: _hook
    mod.set_axon_ntff_profile_hook = lambda h: None
    sys.modules["antenv.axon_hooks"] = mod

    import concourse.bass_utils as bu

    bu.upload_artifacts = lambda tmpdir: str(tmpdir)
    _CACHE["hook_installed"] = True


def kernel(**inputs):
    if "nc" not in _CACHE:
        _CACHE["nc"] = _build_module()
    nc = _CACHE["nc"]

    in_maps = _host_prep(**inputs)
    trace = os.environ.get("KERNEL_TRACE", "0") == "1"
    tmpdir = None
    if trace:
        _install_ntff_hook()
        tmpdir = os.environ.get("KERNEL_TRACE_DIR") or None
    res = run_bass_kernel_spmd(nc, in_maps, core_ids=list(range(NCORES)), trace=trace,
                               tmpdir=tmpdir)
    if trace:
        _CACHE["last_exec_time_ns"] = res.exec_time_ns
        _CACHE["last_results"] = res

    y = np.concatenate([res.results[c]["y8"] for c in range(NCORES)], axis=0)
    return y


# revision 31
# speedup vs baseline: 1.0309x; 1.0227x over previous
"""Trainium2 Bass kernel for BEiT attention block (nn_Beit_9560597201107).

Data-parallel over batch: 64 batches -> 8 NeuronCores x 8 batches each.
Fully transposed dataflow (channels on partitions) so the softmax'd
attention matrix is never transposed on-chip:

  xT = x.T (host)                                  [768, 197]
  qkT[c, n] = sum_k WT[k, c] xT[k, n] + bias       [1536, 197]  (q pre-scaled)
  v[m, d]   = sum_k xT[k, m] WT_v[k, d] + bias     [197, 768]   (natural)
  scT[m, n] = sum_d kT[d, m] qT[d, n]              per head
  eT = exp(scT) * exp_rel_T                        (rel bias via exp-mult)
  po[d, n], sums[n] = sum_m [v|1][m, d] eT[m, n]   (ones col -> row 64 = sums)
  cT = po * broadcast(1/sums)   (PE ones-outer-product broadcast)
  y[n, o] = sum_c cT[c, n] projWT[c, o] + bias

Matmuls run in bfloat16 (fp32 PSUM accumulation, 1 cycle/row at any free
size), except the big qk projection which uses fp8e4 with DoubleRow perf
mode (0.5 cycles/row, half the PE energy -> less power throttling).  fp8
q/k weights are pre-scaled x64 on the host to stay out of the subnormal
range and rescaled by 1/64 in the PSUM->SBUF copy.  The attention
normalization is deferred two head-pair steps and the scores of the next
pair are prefetched so the PE never waits on the exp/reciprocal chains.
"""

import os
import numpy as np
import ml_dtypes

import concourse.bass as bass
import concourse.bacc as bacc
import concourse.mybir as mybir
import concourse.tile as tile
from concourse.bass_utils import run_bass_kernel_spmd
from concourse.bass_interp import get_hw_module

B, N, DIM, HEADS, NBS = 64, 197, 768, 12, 10
HEAD_DIM = DIM // HEADS
SCALE = HEAD_DIM ** -0.5
NCORES = 8
BPC = B // NCORES          # batches per core
KT = DIM // 128            # 6 contraction tiles
NF = N                     # token free-dim, exact (bf16 full rate at any size)
NF2 = 2 * NF
TOK_TILES = [(0, 128), (128, 69)]  # (offset, size) over the 197 tokens
# Heads grouped in same-parity pairs: both heads of a pair live at the same
# 64-partition half of qkT, so their back-to-back matmuls into one PSUM bank
# use the same PE row group.
PAIRS = [(0, 2), (4, 6), (8, 10), (1, 3), (5, 7), (9, 11)]
PAIR_PERM = [h for p in PAIRS for h in p]

F32 = mybir.dt.float32
FP8 = mybir.dt.float8e4
FP8NP = ml_dtypes.float8_e4m3
QS = 64.0  # fp8 weight pre-scale (keeps q/k weights out of subnormal range)
BF16 = mybir.dt.bfloat16
BFNP = ml_dtypes.bfloat16

_CACHE = {}


def _build_module():
    nc = bacc.Bacc("TRN2", target_bir_lowering=False, debug=False)

    # host-transposed x: xt8[b, k, p, n] = x[b, n, 128k+p]
    xt8_d = nc.dram_tensor("xt8", [BPC, 128, KT, NF], BF16, kind="ExternalInput")
    wt_d = nc.dram_tensor("wt", [KT, 128, DIM], BF16, kind="ExternalInput")
    wt8_d = nc.dram_tensor("wt8", [3, 128, 2, 2 * DIM], FP8, kind="ExternalInput")
    xt8f8_d = nc.dram_tensor("xt8f8", [BPC // 2, 128, 3, 2, 2, NF], FP8,
                             kind="ExternalInput")
    pwt_d = nc.dram_tensor("pwt", [128, KT, DIM], BF16, kind="ExternalInput")
    qbc_d = nc.dram_tensor("qbc", [128, BPC, KT], F32, kind="ExternalInput")
    vpb_d = nc.dram_tensor("vpb8", [BPC, 128, DIM], BF16, kind="ExternalInput")
    relt_d = nc.dram_tensor("relt", [128, 6, 2, NF2], BF16, kind="ExternalInput")
    twohot_d = nc.dram_tensor("twohot", [128, 128], BF16, kind="ExternalInput")
    y8_d = nc.dram_tensor("y8", [BPC, N, DIM], F32, kind="ExternalOutput")

    with tile.TileContext(nc) as tc:
        with (
            tc.tile_pool(name="const", bufs=1) as constp,
            tc.tile_pool(name="sb_xT", bufs=4) as sb_xT,
            tc.tile_pool(name="sb_qkT", bufs=2) as sb_qkT,
            tc.tile_pool(name="sb_v", bufs=2) as sb_v,
            tc.tile_pool(name="sb_exp", bufs=2) as sb_exp,
            tc.tile_pool(name="sb_po", bufs=3) as sb_po,
            tc.tile_pool(name="sb_pohi", bufs=3) as sb_pohi,
            tc.tile_pool(name="sb_rec", bufs=6) as sb_rec,
            tc.tile_pool(name="sb_cT", bufs=12) as sb_cT,
            tc.tile_pool(name="sb_out", bufs=2) as sb_out,
            tc.tile_pool(name="sb_vpb", bufs=2) as sb_vpb,
            tc.tile_pool(name="ps", bufs=5, space="PSUM") as ps,
            tc.tile_pool(name="ps_pj", bufs=2, space="PSUM") as ps_pj,
            tc.tile_pool(name="ps_pb", bufs=1, space="PSUM") as ps_pb,
        ):
            # ---- persistent data; wt chunked per k-tile so the first qkT
            # ---- matmuls can start as soon as chunk 0 lands
            qbc_sb = constp.tile([128, BPC, KT], F32)
            nc.sync.dma_start(out=qbc_sb[:], in_=qbc_d.ap())
            wt8_sb = constp.tile([128, 3, 2, 2 * DIM], FP8)
            nc.gpsimd.dma_start(out=wt8_sb[:, 0, :, 0:512],
                                in_=wt8_d.ap()[0][:, :, 0:512])
            nc.gpsimd.dma_start(out=wt8_sb[:, 0, :, 512:2 * DIM],
                                in_=wt8_d.ap()[0][:, :, 512:2 * DIM])
            for k in range(1, 3):
                nc.gpsimd.dma_start(out=wt8_sb[:, k, :, :], in_=wt8_d.ap()[k])
            wt_sb = constp.tile([128, KT, DIM], BF16)
            for k in range(KT):
                nc.gpsimd.dma_start(out=wt_sb[:, k, :], in_=wt_d.ap()[k])
            relt_sb = constp.tile([128, 6, 2, NF2], BF16)
            nc.gpsimd.dma_start(out=relt_sb[:], in_=relt_d.ap())
            pwt_sb = constp.tile([128, KT, DIM], BF16)
            nc.gpsimd.dma_start(out=pwt_sb[:], in_=pwt_d.ap())
            twohot_sb = constp.tile([128, 128], BF16)
            nc.gpsimd.dma_start(out=twohot_sb[:], in_=twohot_d.ap())

            def kT(qkT_sb, h, hb, off, mt):
                base = (h % 2) * 64
                return qkT_sb[base:base + 64, 6 + h // 2,
                              hb * NF + off:hb * NF + off + mt]

            def qT(qkT_sb, h, hb):
                base = (h % 2) * 64
                return qkT_sb[base:base + 64, h // 2, hb * NF:(hb + 1) * NF]

            # ---- projection machinery (generic over prev/self batch) ----
            def make_pstate(b_, cT_, vpb_):
                return {"b": b_, "cT": cT_, "vpb": vpb_, "ps": {}}

            def emit_pj(st, t, js):
                off, mt = TOK_TILES[t]
                if t not in st["ps"]:
                    st["ps"][t] = (
                        ps_pj.tile([128, 512], F32, tag="pj",
                                   name=f"pr_{st['b']}_{t}"),
                        ps_pj.tile([128, 256], F32, tag="pj",
                                   name=f"pr2_{st['b']}_{t}"),
                    )
                pr, pr2 = st["ps"][t]
                for j in js:
                    nc.tensor.matmul(
                        pr[0:mt, :], st["cT"][j][:, off:off + mt],
                        pwt_sb[:, j, 0:512], start=(j == 0), stop=(j == 5),
                    )
                    nc.tensor.matmul(
                        pr2[0:mt, :], st["cT"][j][:, off:off + mt],
                        pwt_sb[:, j, 512:768], start=(j == 0), stop=(j == 5),
                    )

            def emit_pj_fin(st, t):
                off, mt = TOK_TILES[t]
                pr, pr2 = st["ps"][t]
                out_sb = sb_out.tile([128, DIM], F32, tag="out",
                                     name=f"out_{st['b']}_{t}")
                nc.vector.tensor_add(out_sb[0:mt, 0:512], pr[0:mt, :],
                                     st["vpb"][0:mt, 0:512])
                nc.vector.tensor_add(out_sb[0:mt, 512:768], pr2[0:mt, :],
                                     st["vpb"][0:mt, 512:768])
                nc.sync.dma_start(out=y8_d.ap()[st["b"], off:off + mt, :],
                                  in_=out_sb[0:mt, :])

            prev_proj = [None]
            pend_norm = [None]

            def load_pair(gg):
                # x loads emitted one pair ahead, right after a qkT phase,
                # so their sync-queue slots precede the norm DMAs of the
                # intervening attention steps
                xT8 = sb_xT.tile([128, 3, 2, 2, NF], FP8, tag="xT8",
                                 name=f"xT8_{gg}")
                for kk in range(3):
                    nc.sync.dma_start(out=xT8[:, kk, :, :, :],
                                      in_=xt8f8_d.ap()[gg][:, kk])
                xT = sb_xT.tile([128, 2, KT, NF], BF16, tag="xT",
                                name=f"xT_{gg}")
                for hb_ in range(2):
                    nc.sync.dma_start(out=xT[:, hb_, :, :],
                                      in_=xt8_d.ap()[2 * gg + hb_])
                return xT8, xT

            xtiles = {0: load_pair(0)}

            for g in range(BPC // 2):
                xT8_sb, xT_sb = xtiles.pop(g)

                # ---- qkT for both batches; k-outer in two ct-halves so the
                # ---- first matmuls only need wt chunk 0 ----
                qkT_sb = sb_qkT.tile([128, 12, NF2], BF16, tag="qkT", name=f"qkT_{g}")
                for third in range(3):
                    cts = list(range(4 * third, 4 * third + 4))
                    qps = {ct: ps.tile([128, NF2], F32, tag="ps",
                                       name=f"qp_{g}_{ct}") for ct in cts}
                    for k in range(3):
                        for ct in cts:
                            nc.tensor.matmul(
                                qps[ct][:],
                                wt8_sb[:, k, :, ct * 128:(ct + 1) * 128],
                                xT8_sb[:, k, :, :, :],
                                start=(k == 0),
                                stop=(k == 2),
                                perf_mode=mybir.MatmulPerfMode.DoubleRow,
                            )
                    for ct in cts:
                        qp = qps[ct]
                        if ct < 6:
                            for hb in range(2):
                                qbias = qbc_sb[:, 2 * g + hb, ct:ct + 1]
                                dst = qkT_sb[:, ct, hb * NF:(hb + 1) * NF]
                                srcp = qp[:, hb * NF:(hb + 1) * NF]
                                if ct % 2 == 0:
                                    nc.vector.tensor_scalar(
                                        out=dst, in0=srcp, scalar1=1.0 / QS,
                                        scalar2=qbias,
                                        op0=mybir.AluOpType.mult,
                                        op1=mybir.AluOpType.add,
                                    )
                                else:
                                    nc.scalar.activation(
                                        dst, srcp,
                                        mybir.ActivationFunctionType.Identity,
                                        bias=qbias, scale=1.0 / QS,
                                    )
                        else:
                            if ct % 2 == 0:
                                nc.vector.tensor_scalar_mul(
                                    qkT_sb[:, ct, :], qp[:], 1.0 / QS)
                            else:
                                nc.scalar.activation(
                                    qkT_sb[:, ct, :], qp[:],
                                    mybir.ActivationFunctionType.Identity,
                                    scale=1.0 / QS,
                                )

                if g + 1 < BPC // 2:
                    xtiles[g + 1] = load_pair(g + 1)

                for hb in range(2):
                    b = 2 * g + hb
                    is_last = (b == BPC - 1)

                    vpb_t = sb_vpb.tile([128, DIM], BF16, tag="vpb", name=f"vpb_{b}")
                    nc.gpsimd.dma_start(out=vpb_t[:], in_=vpb_d.ap()[b])

                    # ---- v (natural layout, 65-wide head slots, col 64 = 1s) ----
                    v_sb = sb_v.tile([128, 2, HEADS, 65], BF16, tag="v",
                                     name=f"v_{b}")
                    nc.vector.memset(v_sb[:, :, :, 64:65], 1.0)
                    for t, (off, mt) in enumerate(TOK_TILES):
                        vp = ps.tile([128, 512], F32, tag="ps", name=f"vp_{b}_{t}")
                        vp2 = ps.tile([128, 256], F32, tag="ps", name=f"vp2_{b}_{t}")
                        for k in range(KT):
                            xsl = xT_sb[:, hb, k, off:off + mt]
                            nc.tensor.matmul(
                                vp[0:mt, :], xsl, wt_sb[:, k, 0:512],
                                start=(k == 0), stop=(k == KT - 1),
                            )
                            nc.tensor.matmul(
                                vp2[0:mt, :], xsl, wt_sb[:, k, 512:768],
                                start=(k == 0), stop=(k == KT - 1),
                            )
                        # v_sb head axis is in PAIR_PERM order: even head h ->
                        # slot h//2, odd head h -> slot 6 + h//2
                        nc.vector.tensor_copy(
                            v_sb[0:mt, t, :, :].rearrange(
                                "p (par a) c -> p a par c", par=2)[:, 0:4, :, 0:64],
                            vp[0:mt, :].rearrange("p (a par d) -> p a par d",
                                                  par=2, d=HEAD_DIM),
                        )
                        nc.scalar.copy(
                            v_sb[0:mt, t, :, :].rearrange(
                                "p (par a) c -> p a par c", par=2)[:, 4:6, :, 0:64],
                            vp2[0:mt, :].rearrange("p (a par d) -> p a par d",
                                                   par=2, d=HEAD_DIM),
                        )

                    # ---- attention, software-pipelined by one head-pair ----
                    cT_sb = [sb_cT.tile([128, NF], BF16, tag="cT",
                                        name=f"cT_{b}_{j}") for j in range(6)]
                    norm_state = {}
                    exp_tiles = {}
                    self_st = make_pstate(b, cT_sb, vpb_t) if is_last else None

                    def emit_norm(sp, st=None, cT_=None):
                        # pb broadcast + cT multiply for pair sp (deferred one
                        # step so the PE never waits on the reciprocal chain)
                        stt = st if st is not None else norm_state
                        cc = cT_ if cT_ is not None else cT_sb
                        po_full, rec_sb, bb = stt.pop(sp)
                        pb = ps_pb.tile([128, NF], F32, tag="pb",
                                        name=f"pb_{bb}_{sp}")
                        nc.tensor.matmul(
                            pb[0:128, :], twohot_sb[64:66, 0:128],
                            rec_sb[64:66, 0:NF], start=True, stop=True,
                        )
                        nc.vector.tensor_mul(cc[sp][:], po_full[:], pb[:])

                    # flush the previous batch's last normalization now that
                    # the v-phase matmuls cover its reciprocal latency
                    if pend_norm[0] is not None:
                        pstt, pcT = pend_norm[0]
                        for psp in sorted(pstt.keys()):
                            emit_norm(psp, st=pstt, cT_=pcT)
                        pend_norm[0] = None

                    def emit_sc(sp):
                        h0, h1 = PAIRS[sp]
                        expT = sb_exp.tile([128, 2, NF2], BF16, tag="expT",
                                           name=f"expT_{b}_{sp}")
                        scs = []
                        for t, (off, mt) in enumerate(TOK_TILES):
                            sc = ps.tile([128, NF2], F32, tag="ps",
                                         name=f"sc_{b}_{sp}_{t}")
                            nc.tensor.matmul(
                                sc[0:mt, 0:NF], kT(qkT_sb, h0, hb, off, mt),
                                qT(qkT_sb, h0, hb), start=True, stop=True,
                            )
                            nc.tensor.matmul(
                                sc[0:mt, NF:NF2], kT(qkT_sb, h1, hb, off, mt),
                                qT(qkT_sb, h1, hb), start=True, stop=True,
                            )
                            scs.append(sc)
                        for t, (off, mt) in enumerate(TOK_TILES):
                            nc.scalar.activation(
                                expT[0:mt, t, :], scs[t][0:mt, :],
                                mybir.ActivationFunctionType.Exp,
                            )
                            eng = nc.vector if t == 0 else nc.gpsimd
                            eng.tensor_mul(
                                expT[0:mt, t, :], expT[0:mt, t, :],
                                relt_sb[0:mt, sp, t, :],
                            )
                        exp_tiles[sp] = expT

                    # prev-batch proj schedule: normal batches spread 6 chunks
                    # over the 6 steps; the last batch compresses them into
                    # steps 0-3 and starts its own projection early
                    if prev_proj[0] is not None:
                        prev_st = make_pstate(*prev_proj[0])
                        if is_last:
                            sched = {
                                0: [("pj", prev_st, 0, [0, 1, 2, 3])],
                                1: [("pj", prev_st, 0, [4, 5]),
                                    ("fin", prev_st, 0)],
                                2: [("pj", prev_st, 1, [0, 1, 2, 3])],
                                3: [("pj", prev_st, 1, [4, 5]),
                                    ("fin", prev_st, 1),
                                    ("pj", self_st, 0, [0])],
                                4: [("pj", self_st, 0, [1])],
                                5: [("pj", self_st, 0, [2, 3, 4])],
                            }
                        else:
                            sched = {
                                0: [("pj", prev_st, 0, [0, 1])],
                                1: [("pj", prev_st, 0, [2, 3])],
                                2: [("pj", prev_st, 0, [4, 5]),
                                    ("fin", prev_st, 0)],
                                3: [("pj", prev_st, 1, [0, 1])],
                                4: [("pj", prev_st, 1, [2, 3])],
                                5: [("pj", prev_st, 1, [4, 5]),
                                    ("fin", prev_st, 1)],
                            }
                    else:
                        sched = {}

                    emit_sc(0)
                    for sp in range(6):
                        if sp < 5:
                            emit_sc(sp + 1)
                        if sp >= 2 and (sp - 2) in norm_state:
                            emit_norm(sp - 2)
                        if is_last and sp == 5 and 4 in norm_state:
                            emit_norm(4)
                        for item in sched.get(sp, []):
                            if item[0] == "pj":
                                emit_pj(item[1], item[2], item[3])
                            else:
                                emit_pj_fin(item[1], item[2])
                        # one PSUM bank per head: a start=True resets the
                        # whole bank's has-written bits, so accumulation
                        # groups must not interleave within a bank
                        expT = exp_tiles.pop(sp)
                        pos = [ps.tile([65, NF], F32, tag="ps",
                                       name=f"po_{b}_{sp}_{i}") for i in (0, 1)]
                        for i in (0, 1):
                            for t, (off, mt) in enumerate(TOK_TILES):
                                nc.tensor.matmul(
                                    pos[i][0:65, :],
                                    v_sb[0:mt, t, 2 * sp + i, :],
                                    expT[0:mt, t, i * NF:(i + 1) * NF],
                                    start=(t == 0), stop=(t == 1),
                                )
                        # row 64 of po = per-token exp sums (65-col
                        # stationary with a trailing ones column)
                        po_sb = sb_po.tile([66, NF2], F32, tag="po",
                                           name=f"po_sb_{b}_{sp}")
                        nc.vector.tensor_copy(po_sb[0:65, 0:NF], pos[0][0:65, :])
                        nc.scalar.copy(po_sb[0:65, NF:NF2], pos[1][0:65, :])
                        # h1 sums shift to partition 65 so one K=2 two-hot
                        # matmul broadcasts both heads' reciprocals at once
                        nc.sync.dma_start(out=po_sb[65:66, 0:NF],
                                          in_=po_sb[64:65, NF:NF2])
                        # rows 0:64 are po values (garbage out, never read);
                        # the custom DVE op needs base partition 0
                        rec_f = sb_rec.tile([66, NF], F32, tag="recf",
                                            name=f"recf_{b}_{sp}")
                        nc.vector.reciprocal_approx_fast(out=rec_f[0:66, :],
                                                         in_=po_sb[0:66, 0:NF])
                        rec_sb = sb_rec.tile([66, NF], BF16, tag="rec",
                                             name=f"rec_{b}_{sp}")
                        nc.vector.tensor_copy(rec_sb[64:66, :], rec_f[64:66, :])
                        # assemble both heads' po on partitions 0:128 so the
                        # cT normalization is a single 128-partition multiply
                        po_full = sb_pohi.tile([128, NF], F32, tag="pohi",
                                               name=f"pofull_{b}_{sp}")
                        nc.sync.dma_start(out=po_full[0:64, :],
                                          in_=po_sb[0:64, 0:NF])
                        nc.sync.dma_start(out=po_full[64:128, :],
                                          in_=po_sb[0:64, NF:NF2])
                        norm_state[sp] = (po_full, rec_sb, b)

                    if is_last:
                        emit_norm(5)
                        emit_pj(self_st, 0, [5])
                        emit_pj_fin(self_st, 0)
                        emit_pj(self_st, 1, [0, 1, 2, 3, 4, 5])
                        emit_pj_fin(self_st, 1)
                    else:
                        pend_norm[0] = (norm_state, cT_sb)
                        prev_proj[0] = (b, cT_sb, vpb_t)

    nc.compile()
    nc.m = get_hw_module(nc.m)
    return nc


def _host_prep(x, qkv_weight, q_bias, v_bias, rel_table, proj_weight, proj_bias,
               b_idx, rel_index):
    x = np.asarray(x, dtype=np.float32)
    # xt8[b, k, p, n] = x[b, n, 128k+p]
    xt = np.ascontiguousarray(
        x.transpose(0, 2, 1).reshape(B, KT, 128, N)
        .transpose(0, 2, 1, 3)).astype(BFNP)
    # pair-interleaved fp8 x: [pair, p, k, ko, hb, n]
    xc8 = (x.transpose(0, 2, 1).reshape(B // 2, 2, 3, 2, 128, N)
           .transpose(0, 4, 2, 3, 1, 5)).astype(FP8NP)
    xc8 = np.ascontiguousarray(xc8)
    W = np.asarray(qkv_weight, dtype=np.float32).copy()
    W[:DIM] *= np.float32(SCALE)
    # v-only bf16 weights
    wt = np.ascontiguousarray(W[2 * DIM:].T.reshape(KT, 128, DIM)).astype(BFNP)
    # q,k weights in fp8, pre-scaled by QS; c = 256k + 128ko + p
    wqk = (W[0:2 * DIM] * np.float32(QS)).T  # [768c, 1536m]
    wt8 = np.ascontiguousarray(
        wqk.reshape(3, 2, 128, 2 * DIM).transpose(0, 2, 1, 3)).astype(FP8NP)
    pwtT = np.asarray(proj_weight, dtype=np.float32).T  # [c', o]
    pwtT = pwtT.reshape(HEADS, HEAD_DIM, DIM)[PAIR_PERM].reshape(DIM, DIM)
    pwt = np.ascontiguousarray(
        pwtT.reshape(KT, 128, DIM).transpose(1, 0, 2)).astype(BFNP)

    bi = np.asarray(b_idx).astype(np.int64)
    qb_all = (np.asarray(q_bias, dtype=np.float32)[bi] * np.float32(SCALE))
    vb_all = np.asarray(v_bias, dtype=np.float32)[bi]
    # softmax rows sum to 1, so attn @ (1 x vb) == 1 x vb; push the v bias
    # through the projection into the proj bias
    pb_all = (np.asarray(proj_bias, dtype=np.float32)[bi]
              + vb_all @ np.asarray(proj_weight, dtype=np.float32).T).astype(BFNP)
    pb_bcast = np.ascontiguousarray(
        np.broadcast_to(pb_all[:, None, :], (B, 128, DIM)))

    ridx = np.asarray(rel_index).astype(np.int64)
    rel = np.asarray(rel_table, dtype=np.float32)[ridx.reshape(-1)]
    rel = rel.reshape(N, N, HEADS)  # [n, m, h]
    relth = np.zeros((HEADS, 2, 128, NF), dtype=np.float32)
    for t, (off, mt) in enumerate(TOK_TILES):
        # relth[h, t, p, n] = exp(rel[n, off+p, h])
        relth[:, t, 0:mt, :] = np.exp(rel[:, off:off + mt, :].transpose(2, 1, 0))
    # pair-merged: relt[sp, t, p, i*NF+n] = relth[PAIRS[sp][i], t, p, n]
    relt = np.ascontiguousarray(
        relth[PAIR_PERM].reshape(6, 2, 2, 128, NF)
        .transpose(0, 2, 3, 1, 4).reshape(6, 2, 128, NF2)
        .transpose(2, 0, 1, 3)).astype(BFNP)

    twohot = np.zeros((128, 128), dtype=BFNP)
    twohot[64, 0:64] = 1.0
    twohot[65, 64:128] = 1.0

    in_maps = []
    for c in range(NCORES):
        sl = slice(c * BPC, (c + 1) * BPC)
        qbc = np.ascontiguousarray(
            qb_all[sl].reshape(BPC, KT, 128).transpose(2, 0, 1))
        vpb = np.ascontiguousarray(pb_bcast[sl])
        in_maps.append({
            "xt8": np.ascontiguousarray(xt[sl]),
            "xt8f8": np.ascontiguousarray(xc8[c * (BPC // 2):(c + 1) * (BPC // 2)]),
            "wt8": wt8,
            "wt": wt,
            "pwt": pwt,
            "qbc": qbc,
            "vpb8": vpb,
            "relt": relt,
            "twohot": twohot,
        })
    return in_maps


def _install_ntff_hook():
    """Provide antenv.axon_hooks (absent from this image) so bass_utils can
    capture NTFF profiles through libaxon_pjrt.so, and keep artifacts local."""
    if _CACHE.get("hook_installed"):
        return
    import sys
    import types
    import ctypes
    import contextlib

    so_path = "/opt/axon/libaxon_pjrt.so"
    lib = ctypes.CDLL(so_path)
    lib.axon_start_nrt_profile.argtypes = [
        ctypes.POINTER(ctypes.c_int64),
        ctypes.c_size_t,
    ]
    lib.axon_start_nrt_profile.restype = ctypes.c_int64
    lib.axon_stop_nrt_profile.argtypes = [ctypes.c_char_p]
    lib.axon_stop_nrt_profile.restype = ctypes.c_int64

    @contextlib.contextmanager
    def _hook(output_dir, device_ids):
        import jax

        jax.devices()
        if device_ids:
            ids = (ctypes.c_int64 * len(device_ids))(*device_ids)
            rc = lib.axon_start_nrt_profile(ids, len(device_ids))
        else:
            rc = lib.axon_start_nrt_profile(None, 0)
        if rc != 0:
            raise RuntimeError(f"axon_start_nrt_profile rc={rc}")
        try:
            yield
        finally:
            n = lib.axon_stop_nrt_profile(str(output_dir).encode())
            print(f"ntff profile: {n} file(s) written to {output_dir}")

    mod = types.ModuleType("antenv.axon_hooks")
    mod.get_axon_ntff_profile_hook = lambda: _hook
    mod.set_axon_ntff_profile_hook = lambda h: None
    sys.modules["antenv.axon_hooks"] = mod

    import concourse.bass_utils as bu

    bu.upload_artifacts = lambda tmpdir: str(tmpdir)
    _CACHE["hook_installed"] = True


def kernel(**inputs):
    if "nc" not in _CACHE:
        _CACHE["nc"] = _build_module()
    nc = _CACHE["nc"]

    in_maps = _host_prep(**inputs)
    trace = os.environ.get("KERNEL_TRACE", "0") == "1"
    tmpdir = None
    if trace:
        _install_ntff_hook()
        tmpdir = os.environ.get("KERNEL_TRACE_DIR") or None
    res = run_bass_kernel_spmd(nc, in_maps, core_ids=list(range(NCORES)), trace=trace,
                               tmpdir=tmpdir)
    if trace:
        _CACHE["last_exec_time_ns"] = res.exec_time_ns
        _CACHE["last_results"] = res

    y = np.concatenate([res.results[c]["y8"] for c in range(NCORES)], axis=0)
    return y
